# revision 8
# baseline (speedup 1.0000x reference)
"""Multi-head causal attention (B=4, S=2048, D=1024, H=16) on 8 TRN2 NeuronCores.

Sharding: 2 cores per batch element, 8 heads (512 dims) per core.
Each core computes QKV projections for its head slice, causal attention,
and a partial output projection (its 512 rows of Wo). The host sums the
two partial outputs per batch element (the tensor-parallel all-reduce,
folded into the gather step).

Compute dtype: bf16 matmul inputs with fp32 PSUM accumulation (weights
and activations converted to bf16 host-side / on write-back).

Per-core dataflow (layouts chosen so no activation needs a transpose
after the initial X^T build, which itself is a DMA transpose):
  1. X^T [d x seq] chunks via DMA transpose (bf16).
  2. Q^T, K^T [dim(512) x seq] = W^T @ X^T, V [seq x dim] = X @ Wv,
     V stored bf16 with a ones column appended (the ones column makes
     the P@V matmul also emit softmax row sums).
  3. Per head: S^T tiles [ks,qs] = K @ Q^T, exp on ACT -> bf16 P^T,
     causal mask multiply on the 4 diagonal tiles (in-tile triangle
     only spans the first 128 columns), ctx^T accumulated as
     V_aug^T @ P^T (no P transpose needed in this orientation).
     Score matmuls for block N and ctx matmuls for block N-1 are
     emitted interleaved per k-tile so the in-order PE never stalls
     on the ACT exp drain.
     Normalize with reciprocal of the sums row (fast NR-approx),
     broadcast across partitions via gpsimd.  bv added
     post-normalization (softmax rows sum to 1, so folding bv there
     is exact).
  4. out_partial = ctx^T.T @ Wo (+ bo on even cores only).
"""

import sys

import numpy as np


def _ensure_concourse():
    try:
        import concourse  # noqa: F401
    except ImportError:
        sys.path.insert(0, "/opt/trn_rl_repo")


_ensure_concourse()

B, S, D, H, HD = 4, 2048, 1024, 16, 64
DC = 512  # dims (= 8 heads) per core
N_CORES = 8

_nc_cache = None


def _build_bass():
    from contextlib import ExitStack

    import concourse.mybir as mybir
    import concourse.tile as tile
    from concourse import bacc

    f32 = mybir.dt.float32
    bf16 = mybir.dt.bfloat16
    Exp = mybir.ActivationFunctionType.Exp

    nc = bacc.Bacc(None, target_bir_lowering=False)

    x = nc.dram_tensor("x", [S, D], bf16, kind="ExternalInput")
    wq = nc.dram_tensor("wq", [D, DC], bf16, kind="ExternalInput")
    wk = nc.dram_tensor("wk", [D, DC], bf16, kind="ExternalInput")
    wv = nc.dram_tensor("wv", [D, DC], bf16, kind="ExternalInput")
    wo = nc.dram_tensor("wo", [DC, D], bf16, kind="ExternalInput")
    bq_d = nc.dram_tensor("bq", [128, 4], f32, kind="ExternalInput")
    bk_d = nc.dram_tensor("bk", [128, 4], f32, kind="ExternalInput")
    bv_d = nc.dram_tensor("bv", [128, 4], f32, kind="ExternalInput")
    bo_d = nc.dram_tensor("bo", [1, D], bf16, kind="ExternalInput")
    out = nc.dram_tensor("out", [S, D], f32, kind="ExternalOutput")

    wq_r = wq[:, :].rearrange("(ko ki) n -> ki ko n", ki=128)  # [128,8,512]
    wk_r = wk[:, :].rearrange("(ko ki) n -> ki ko n", ki=128)
    wv_r = wv[:, :].rearrange("(ko ki) n -> ki ko n", ki=128)
    wo_r = wo[:, :].rearrange("(ko ki) n -> ki ko n", ki=128)  # [128,4,1024]
    our = out[:, :].rearrange("(so si) d -> si so d", si=128)

    with tile.TileContext(nc) as tc, ExitStack() as ctx:
        pers = ctx.enter_context(tc.tile_pool(name="pers", bufs=1))
        qt = pers.tile([128, 4, S], bf16, name="qt")  # Q^T: dim x seq
        ktt = pers.tile([128, 4, S], bf16, name="ktt")  # K^T: dim x seq
        vaug = pers.tile([128, 16, 8, 65], bf16, name="vaug")  # V + ones col
        ones_row = pers.tile([1, 128], bf16, name="ones_row")
        bo_bc = pers.tile([128, D], f32, name="bo_bc")
        bo_row = pers.tile([1, D], bf16, name="bo_row")
        bq_sb = pers.tile([128, 4], f32, name="bq_sb")
        bk_sb = pers.tile([128, 4], f32, name="bk_sb")
        bv_sb = pers.tile([128, 4], f32, name="bv_sb")

        # ---- constants / small inputs ----
        nc.vector.memset(ones_row, 1.0)
        nc.gpsimd.memset(vaug[:, :, :, 64:65], 1.0)
        nc.sync.dma_start(bq_sb[:, :], bq_d[:, :])
        nc.sync.dma_start(bk_sb[:, :], bk_d[:, :])
        nc.sync.dma_start(bv_sb[:, :], bv_d[:, :])
        nc.sync.dma_start(bo_row[:, :], bo_d[:, :])

        # bo broadcast across partitions via ones-vector matmul
        with tc.tile_pool(name="initps", bufs=2, space="PSUM") as initps:
            for nb in range(2):
                pb = initps.tile([128, 512], f32, tag="initp")
                nc.tensor.matmul(
                    pb,
                    lhsT=ones_row[:, :],
                    rhs=bo_row[:, nb * 512 : (nb + 1) * 512],
                    start=True,
                    stop=True,
                )
                nc.any.tensor_copy(bo_bc[:, nb * 512 : (nb + 1) * 512], pb)

        # ---- fused pipeline ----
        # Query block qb's attention needs K/V/Q only for seq chunks <= qb
        # (causal), so QKV projection of chunk qb is emitted immediately
        # before attention on block qb.  This interleaves the PE-heavy
        # projection work with the ACT-heavy exp work of earlier blocks.
        late = ctx.enter_context(tc.tile_pool(name="late", bufs=1))
        ctxT = late.tile([128, 4, S], bf16, name="ctxT")
        wo_sb = late.tile([128, 4, D], bf16, name="wo_sb")
        wq_sb = late.tile([128, 8, DC], bf16, name="wq_sb")
        wk_sb = late.tile([128, 8, DC], bf16, name="wk_sb")
        wv_sb = late.tile([128, 8, DC], bf16, name="wv_sb")
        with (
            tc.tile_pool(name="xt", bufs=3) as xt_pool,
            tc.tile_pool(name="ptp", bufs=32) as pt_pool,
            tc.tile_pool(name="pps", bufs=2, space="PSUM") as pps,
            tc.tile_pool(name="sps", bufs=2, space="PSUM") as sps,
            tc.tile_pool(name="ups", bufs=2, space="PSUM") as ups,
            tc.tile_pool(name="smp", bufs=2) as smp,
            tc.tile_pool(name="osb", bufs=3) as osb_pool,
        ):
            def emit_xt(sb):
                """X^T DMA transposes for seq chunk sb."""
                ssl = slice(sb * 512, (sb + 1) * 512)
                xt_chunk = xt_pool.tile([128, 8, 512], bf16, tag="xt")
                for kd in range(8):
                    nc.sync.dma_start_transpose(
                        xt_chunk[:, kd, :], x[ssl, kd * 128 : (kd + 1) * 128]
                    )
                return xt_chunk

            def emit_qkv_chunk(sb, xt_chunk):
                """Q/K/V projections for seq chunk sb."""
                ssl = slice(sb * 512, (sb + 1) * 512)
                for m in range(4):  # output dim tiles (heads 2m, 2m+1)
                    pq = pps.tile([128, 512], f32, tag="pj")
                    for kd in range(8):
                        nc.tensor.matmul(
                            pq,
                            lhsT=wq_sb[:, kd, m * 128 : (m + 1) * 128],
                            rhs=xt_chunk[:, kd, :],
                            start=(kd == 0),
                            stop=(kd == 7),
                        )
                    nc.vector.tensor_scalar_add(
                        qt[:, m, ssl], pq, bq_sb[:, m : m + 1]
                    )
                    pk = pps.tile([128, 512], f32, tag="pj")
                    for kd in range(8):
                        nc.tensor.matmul(
                            pk,
                            lhsT=wk_sb[:, kd, m * 128 : (m + 1) * 128],
                            rhs=xt_chunk[:, kd, :],
                            start=(kd == 0),
                            stop=(kd == 7),
                        )
                    nc.vector.tensor_scalar_add(
                        ktt[:, m, ssl], pk, bk_sb[:, m : m + 1]
                    )
                for sv in range(4):  # V rows for this chunk (no bias here)
                    pv = pps.tile([128, 512], f32, tag="pj")
                    for kd in range(8):
                        nc.tensor.matmul(
                            pv,
                            lhsT=xt_chunk[:, kd, sv * 128 : (sv + 1) * 128],
                            rhs=wv_sb[:, kd, :],
                            start=(kd == 0),
                            stop=(kd == 7),
                        )
                    nc.vector.tensor_copy(
                        vaug[:, sb * 4 + sv, :, 0:64],
                        pv[:, :].rearrange("p (h i) -> p h i", h=8),
                    )

            def emit_score_tile(hp, qb, kti):
                """Score matmul pair + exp for one (head-pair, q-block, k-tile).

                Both heads of the pair go into one [128, 2, 512] PSUM tile
                (2 banks) so a single ACT exp covers them; the two matmuls
                row-tile ((0,0)/(64,0)) and run concurrently in the array.
                Diagonal k-tiles compute only their valid query columns; the
                in-tile triangle (which only spans the first 128 columns) is
                zeroed with affine_select (valid iff p <= local f) directly
                on the bf16 P^T tile.
                """
                oi = kti - 4 * qb
                qoff = max(oi, 0) * 128
                w = 512 - qoff
                ps = sps.tile([128, 2, 512], f32, tag="s")
                for h2 in range(2):
                    base = h2 * 64
                    nc.tensor.matmul(
                        ps[:, h2, :w],
                        lhsT=ktt[
                            base : base + 64, hp, kti * 128 : (kti + 1) * 128
                        ],
                        rhs=qt[
                            base : base + 64, hp,
                            qb * 512 + qoff : (qb + 1) * 512,
                        ],
                        start=True,
                        stop=True,
                    )
                p_t = pt_pool.tile([128, 2, 512], bf16, tag="p")
                nc.scalar.activation(p_t[:, :, :w], ps[:, :, :w], Exp, scale=0.125)
                if oi >= 0:
                    mw = min(128, w)
                    nc.gpsimd.affine_select(
                        out=p_t[:, :, :mw],
                        in_=p_t[:, :, :mw],
                        compare_op=mybir.AluOpType.is_ge,
                        fill=0.0,
                        base=0,
                        channel_multiplier=-1,
                        pattern=[[0, 2], [1, mw]],
                    )
                return (kti, qoff, w, p_t)

            def emit_ctx_mm(hp, u_pair, tile_info, first, last):
                """P^T @ V accumulation matmuls for one k-tile of a block."""
                kti, qoff, w, p_t = tile_info
                for h2 in range(2):
                    nc.tensor.matmul(
                        u_pair[h2][:, qoff : qoff + w],
                        lhsT=vaug[:, kti, 2 * hp + h2, :],
                        rhs=p_t[:, h2, :w],
                        start=first,
                        stop=last,
                    )

            def emit_ctx_norm(hp, qb, u_pair):
                """Softmax normalization + bias for a completed block."""
                qsl = slice(qb * 512, (qb + 1) * 512)
                for h2 in range(2):
                    base = h2 * 64
                    u = u_pair[h2]
                    # Evacuate PSUM immediately (cheap copies) so the next
                    # block's ctx matmuls get the bank back; the slow
                    # reciprocal then runs off the critical path on SBUF.
                    sums = smp.tile([1, 512], f32, tag="sums")
                    nc.vector.tensor_copy(sums, u[64:65, :])
                    craw = smp.tile([64, 512], bf16, tag="craw")
                    with nc.allow_low_precision(
                        reason="unnormalized ctx rounded to bf16 pre-divide; "
                        "~0.4% rel, within tolerance"
                    ):
                        nc.vector.tensor_copy(craw, u[0:64, :])
                    rec = smp.tile([1, 512], bf16, tag="rec")
                    with nc.allow_low_precision(
                        reason="softmax 1/sum rounded to bf16; ~0.4% rel, "
                        "within tolerance"
                    ):
                        nc.vector.reciprocal(rec, sums)
                    pb_sb = smp.tile([64, 512], bf16, tag="pbs")
                    nc.gpsimd.partition_broadcast(pb_sb[:, :], rec[:, :])
                    dst = ctxT[base : base + 64, hp, qsl]
                    nc.vector.tensor_mul(dst, craw, pb_sb)
                    nc.vector.tensor_scalar_add(
                        dst, dst, bv_sb[base : base + 64, hp : hp + 1]
                    )

            def emit_fused(cur, prev):
                """Interleave score tiles of `cur` with ctx matmuls of `prev`
                so the in-order PE alternates score and ctx work instead of
                stalling on the ACT exp drain."""
                tiles_new = []
                n_new = 4 * cur[1] + 4 if cur is not None else 0
                if prev is not None:
                    hp_p, qb_p, tiles_p = prev
                    u_pair = [
                        ups.tile([65, 512], f32, tag="u", name=f"u{h2}")
                        for h2 in range(2)
                    ]
                    n_prev = len(tiles_p)
                else:
                    n_prev = 0
                for i in range(max(n_new, n_prev)):
                    if i < n_prev:
                        emit_ctx_mm(
                            hp_p, u_pair, tiles_p[i], i == 0, i == n_prev - 1
                        )
                    if i < n_new:
                        tiles_new.append(emit_score_tile(cur[0], cur[1], i))
                if prev is not None:
                    emit_ctx_norm(hp_p, qb_p, u_pair)
                return tiles_new

            def emit_outproj(qb):
                """Output projection for the 4 seq tiles of query block qb."""
                for ms in range(qb * 4, qb * 4 + 4):
                    for nb in range(2):
                        po = pps.tile([128, 512], f32, tag="pj")
                        for kd in range(4):
                            nc.tensor.matmul(
                                po,
                                lhsT=ctxT[:, kd, ms * 128 : (ms + 1) * 128],
                                rhs=wo_sb[:, kd, nb * 512 : (nb + 1) * 512],
                                start=(kd == 0),
                                stop=(kd == 3),
                            )
                        ot = osb_pool.tile([128, 512], f32, tag="ot")
                        nc.vector.tensor_add(
                            ot, po, bo_bc[:, nb * 512 : (nb + 1) * 512]
                        )
                        nc.sync.dma_start(
                            our[:, ms, nb * 512 : (nb + 1) * 512], ot
                        )

            # Software pipeline: block N's scores are emitted interleaved
            # with block N-1's ctx matmuls.  QKV for chunk qb is emitted
            # right before the attention blocks that first need it, and
            # the output projection for a query block follows its last
            # head-pair.  Weight DMAs are split per contraction slice so
            # the first projection matmuls can start before the full
            # weight tensors land.
            prev = None
            xt_next = emit_xt(0)
            for kd in range(8):
                nc.sync.dma_start(wq_sb[:, kd, :], wq_r[:, kd, :])
                nc.sync.dma_start(wk_sb[:, kd, :], wk_r[:, kd, :])
                nc.sync.dma_start(wv_sb[:, kd, :], wv_r[:, kd, :])
            for qb in range(4):
                xt_chunk = xt_next
                emit_qkv_chunk(qb, xt_chunk)
                if qb == 0:
                    nc.sync.dma_start(wo_sb[:, :, :], wo_r)
                for hp in range(4):
                    tiles = emit_fused((hp, qb), prev)
                    prev_done = prev
                    prev = (hp, qb, tiles)
                    if prev_done is not None and prev_done[0] == 3:
                        emit_outproj(prev_done[1])
                    if hp == 0 and qb < 3:
                        # prefetch next chunk's X^T while attention runs
                        xt_next = emit_xt(qb + 1)
            emit_fused(None, prev)
            emit_outproj(3)

    nc.finalize()
    return nc


def _get_nc():
    global _nc_cache
    if _nc_cache is None:
        _nc_cache = _build_bass()
    return _nc_cache


def make_in_maps(inputs, Wq, bq, Wk, bk, Wv, bv, Wo, bo):
    import ml_dtypes

    bf = ml_dtypes.bfloat16
    inputs = np.asarray(inputs, dtype=np.float32)
    Wq, Wk, Wv, Wo = (np.asarray(a, dtype=np.float32) for a in (Wq, Wk, Wv, Wo))
    bq, bk, bv, bo = (np.asarray(a, dtype=np.float32) for a in (bq, bk, bv, bo))
    in_maps = []
    for c in range(N_CORES):
        b = c // 2
        lo = (c % 2) * DC
        hi = lo + DC
        in_maps.append(
            {
                "x": np.ascontiguousarray(inputs[b]).astype(bf),
                "wq": np.ascontiguousarray(Wq[:, lo:hi]).astype(bf),
                "wk": np.ascontiguousarray(Wk[:, lo:hi]).astype(bf),
                "wv": np.ascontiguousarray(Wv[:, lo:hi]).astype(bf),
                "wo": np.ascontiguousarray(Wo[lo:hi, :]).astype(bf),
                "bq": np.ascontiguousarray(bq[lo:hi].reshape(4, 128).T),
                "bk": np.ascontiguousarray(bk[lo:hi].reshape(4, 128).T),
                "bv": np.ascontiguousarray(bv[lo:hi].reshape(4, 128).T),
                "bo": (
                    bo.reshape(1, D).astype(bf)
                    if c % 2 == 0
                    else np.zeros((1, D), dtype=bf)
                ),
            }
        )
    return in_maps


def run(in_maps, trace=False):
    from concourse.bass_utils import run_bass_kernel_spmd

    nc = _get_nc()
    res = run_bass_kernel_spmd(
        nc, in_maps, core_ids=list(range(N_CORES)), trace=trace
    )
    parts = [r["out"] for r in res.results]
    full = np.stack(
        [parts[2 * b] + parts[2 * b + 1] for b in range(B)]
    ).astype(np.float32)
    return full, res


def kernel(inputs, Wq, bq, Wk, bk, Wv, bv, Wo, bo):
    in_maps = make_in_maps(inputs, Wq, bq, Wk, bk, Wv, bv, Wo, bo)
    full, _ = run(in_maps, trace=False)
    return full
